# revision 15
# baseline (speedup 1.0000x reference)
"""Trainium2 Bass kernel for nn_BeliefPlausibilityFocused.

reference():
    cardinal_fod = inputs.shape[-1] - 1 = 3; n_sets = 8
    bel[..., j] = 1.0 if (j & focal) == focal else 0.0
    pl[...,  j] = 1.0 if (j & focal) >  0    else 0.0
Both outputs are per-pixel broadcast constants of shape
inputs.shape[:-1] + (8,) = [8, 384, 1248, 8]; the input VALUES are unused.

Strategy (pure data-parallel over batch, per sharding hint):
  - 8 cores, one batch element each. Per-core output: bel/pl each
    [384, 1248, 8] f32 = 15.3 MB -> 30.7 MB of HBM writes per core; no
    inputs are transferred to the device at all.
  - The masks (derived from `focal` on the host) are baked into the program
    as memsets: each 8-periodic pattern is built in a small SBUF tile
    (bulk memset of the majority value + one strided memset per minority
    channel, folded to the mask's minimal period). Each output is then
    written by ONE large HWDGE DMA whose source AP repeats the small tile
    via a stride-0 dim (bel on the SP ring, pl on the ACT ring).
  - Measured ~87 us/core when HBM is uncontended (~410 GB/s store BW,
    ~94% of the 435 GB/s SBUF-port ceiling); all-core aggregate sits at
    the device HBM write roofline (~245 MB over ~85 us).
"""

import numpy as np

import concourse.bacc as bacc
import concourse.mybir as mybir
import concourse.tile as tile
from concourse.bass_utils import run_bass_kernel_spmd

# Problem shapes (hardcoded per contract: kernel.py must be self-contained).
B, H, W, C = 8, 384, 1248, 4
NSETS = 1 << (C - 1)          # 8
N_CORES = 8
P = 128                        # SBUF partitions

PIX = H * W                    # 479232 pixels per batch element
PER_OUT = PIX * NSETS          # 3,833,856 f32 per output per core
PER_PART = PER_OUT // P        # 29,952 f32 per partition
SRC_F = 1248                   # source tile width; 4992 B per repeat chunk
REP = PER_PART // SRC_F        # 24 stride-0 repeats per store

assert PER_OUT % P == 0 and PER_PART % NSETS == 0 and SRC_F % NSETS == 0
assert SRC_F * REP == PER_PART

_NC_CACHE = {}
LAST_RESULTS = None  # BassKernelResults of the most recent run (for test.py)


def _build_nc(bel_mask, pl_mask, src_f=SRC_F):
    """One SPMD program: memset-build the two 8-float mask patterns in SBUF,
    then store each output with one big stride-0-source DMA. The mask values
    (derived from `focal` on the host) are baked into the program."""
    rep = PER_PART // src_f
    assert src_f * rep == PER_PART and src_f % NSETS == 0
    nc = bacc.Bacc(None, target_bir_lowering=False)

    bel = nc.dram_tensor("bel", [P, PER_PART], mybir.dt.float32,
                         kind="ExternalOutput")
    pl = nc.dram_tensor("pl", [P, PER_PART], mybir.dt.float32,
                        kind="ExternalOutput")

    with tile.TileContext(nc) as tc:
        with tc.tile_pool(name="sbuf", bufs=1) as pool:
            belt = pool.tile([P, src_f], mybir.dt.float32, tag="belt")
            plt = pool.tile([P, src_f], mybir.dt.float32, tag="plt")

            # Build each 8-periodic pattern: bulk memset of the majority
            # value, then strided memsets for the minority channels. The
            # mask is folded to its minimal period (e.g. focal=3 masks have
            # period 4) to minimize the strided-memset count.
            # belt on DVE, plt on GpSimd so they fill in parallel.
            for t, mask, eng in ((belt, bel_mask, nc.vector),
                                 (plt, pl_mask, nc.gpsimd)):
                mask = np.asarray(mask)
                q = NSETS
                for cand in (1, 2, 4):
                    if cand < NSETS and np.array_equal(
                            np.tile(mask[:cand], NSETS // cand), mask):
                        q = cand
                        break
                pm = mask[:q]
                ones = [int(c) for c in np.nonzero(pm)[0]]
                zeros = [c for c in range(q) if c not in ones]
                maj, minority = (1.0, zeros) if len(ones) >= len(zeros) \
                    else (0.0, ones)
                eng.memset(t[:], maj)
                t3 = t[:].rearrange("p (r c) -> p r c", c=q)
                for c in minority:
                    # integer index -> squeezed 2D strided AP (3D count-1
                    # APs hard-fault the engines)
                    eng.memset(t3[:, :, c], 1.0 - maj)

            # One store per output; the source AP repeats the small tile via
            # a stride-0 dim. bel on the SP ring, pl on the ACT ring.
            for t, out, eng in ((belt, bel, nc.sync), (plt, pl, nc.scalar)):
                o3 = out[:].rearrange("p (r f) -> p r f", r=rep)
                sap = t[:].unsqueeze(1).broadcast_to([P, rep, src_f])
                eng.dma_start(out=o3, in_=sap)

    nc.finalize()
    return nc


def _get_nc(bel_mask, pl_mask):
    key = (tuple(bel_mask), tuple(pl_mask))
    if key not in _NC_CACHE:
        _NC_CACHE[key] = _build_nc(bel_mask, pl_mask)
    return _NC_CACHE[key]


def kernel(inputs, focal):
    global LAST_RESULTS
    inputs = np.asarray(inputs)
    focal_i = int(np.asarray(focal))
    assert inputs.shape == (B, H, W, C), inputs.shape

    # Host-side mask computation (cheap: 8 elements).
    j = np.arange(NSETS, dtype=np.int64)
    contain = j & focal_i
    bel_mask = (contain == focal_i).astype(np.float32)
    pl_mask = (contain > 0).astype(np.float32)

    nc = _get_nc(bel_mask, pl_mask)
    in_maps = [{} for _ in range(N_CORES)]
    res = run_bass_kernel_spmd(nc, in_maps, list(range(N_CORES)))
    LAST_RESULTS = res

    out_dtype = inputs.dtype
    bel_full = np.empty((B, H, W, NSETS), dtype=out_dtype)
    pl_full = np.empty((B, H, W, NSETS), dtype=out_dtype)
    for b in range(N_CORES):
        bel_full[b] = res.results[b]["bel"].reshape(H, W, NSETS)
        pl_full[b] = res.results[b]["pl"].reshape(H, W, NSETS)
    return (bel_full, pl_full)


# revision 16
# speedup vs baseline: 1.1923x; 1.1923x over previous
"""Trainium2 Bass kernel for nn_BeliefPlausibilityFocused.

reference():
    cardinal_fod = inputs.shape[-1] - 1 = 3; n_sets = 8
    bel[..., j] = 1.0 if (j & focal) == focal else 0.0
    pl[...,  j] = 1.0 if (j & focal) >  0    else 0.0
Both outputs are per-pixel broadcast constants of shape
inputs.shape[:-1] + (8,) = [8, 384, 1248, 8]; the input VALUES are unused.

Strategy (pure data-parallel over batch, per sharding hint):
  - 8 cores, one batch element each. Per-core output: bel/pl each
    [384, 1248, 8] f32 = 15.3 MB -> 30.7 MB of HBM writes per core; no
    inputs are transferred to the device at all.
  - The masks (derived from `focal` on the host) are baked into the program
    as memsets: each 8-periodic pattern is built in a small SBUF tile
    (bulk memset of the majority value + one strided memset per minority
    channel, folded to the mask's minimal period). Each output is then
    written by ONE large HWDGE DMA whose source AP repeats the small tile
    via a stride-0 dim (bel on the SP ring, pl on the ACT ring).
  - Measured ~87 us/core when HBM is uncontended (~410 GB/s store BW,
    ~94% of the 435 GB/s SBUF-port ceiling); all-core aggregate sits at
    the device HBM write roofline (~245 MB over ~85 us).
"""

import sys
import types

import numpy as np

import concourse.bacc as bacc
import concourse.mybir as mybir
import concourse.tile as tile
from concourse.bass_utils import run_bass_kernel_spmd


def _install_ntff_hook_shim():
    """bass_utils imports antenv.axon_hooks when BASS_TRACE=1 under axon, but
    the agent image's antenv package lacks that module (a bare import error
    would crash the run). Provide it, wiring the ctypes NTFF hook when the
    axon .so supports it, else degrading to no tracing."""
    if "antenv.axon_hooks" in sys.modules:
        return
    mod = types.ModuleType("antenv.axon_hooks")
    _slot = [None]
    mod.set_axon_ntff_profile_hook = lambda h: _slot.__setitem__(0, h)
    mod.get_axon_ntff_profile_hook = lambda: _slot[0]
    sys.modules["antenv.axon_hooks"] = mod
    try:
        import antenv

        antenv.axon_hooks = mod
    except Exception:
        pass
    try:
        from trn_agent_boot.trn_boot import _ntff_profile_via_ctypes

        hook = _ntff_profile_via_ctypes("/opt/axon/libaxon_pjrt.so")
        if hook is not None:
            mod.set_axon_ntff_profile_hook(hook)
    except Exception:
        pass  # no profiling available; execution still works


_install_ntff_hook_shim()

# Problem shapes (hardcoded per contract: kernel.py must be self-contained).
B, H, W, C = 8, 384, 1248, 4
NSETS = 1 << (C - 1)          # 8
N_CORES = 8
P = 128                        # SBUF partitions

PIX = H * W                    # 479232 pixels per batch element
PER_OUT = PIX * NSETS          # 3,833,856 f32 per output per core
PER_PART = PER_OUT // P        # 29,952 f32 per partition
SRC_F = 1248                   # source tile width; 4992 B per repeat chunk
REP = PER_PART // SRC_F        # 24 stride-0 repeats per store

assert PER_OUT % P == 0 and PER_PART % NSETS == 0 and SRC_F % NSETS == 0
assert SRC_F * REP == PER_PART

_NC_CACHE = {}
LAST_RESULTS = None  # BassKernelResults of the most recent run (for test.py)


def _build_nc(bel_mask, pl_mask, src_f=SRC_F):
    """One SPMD program: memset-build the two 8-float mask patterns in SBUF,
    then store each output with one big stride-0-source DMA. The mask values
    (derived from `focal` on the host) are baked into the program."""
    rep = PER_PART // src_f
    assert src_f * rep == PER_PART and src_f % NSETS == 0
    nc = bacc.Bacc(None, target_bir_lowering=False)

    bel = nc.dram_tensor("bel", [P, PER_PART], mybir.dt.float32,
                         kind="ExternalOutput")
    pl = nc.dram_tensor("pl", [P, PER_PART], mybir.dt.float32,
                        kind="ExternalOutput")

    with tile.TileContext(nc) as tc:
        with tc.tile_pool(name="sbuf", bufs=1) as pool:
            belt = pool.tile([P, src_f], mybir.dt.float32, tag="belt")
            plt = pool.tile([P, src_f], mybir.dt.float32, tag="plt")

            # Build each 8-periodic pattern: bulk memset of the majority
            # value, then strided memsets for the minority channels. The
            # mask is folded to its minimal period (e.g. focal=3 masks have
            # period 4) to minimize the strided-memset count.
            # belt on DVE, plt on GpSimd so they fill in parallel.
            for t, mask, eng in ((belt, bel_mask, nc.vector),
                                 (plt, pl_mask, nc.gpsimd)):
                mask = np.asarray(mask)
                q = NSETS
                for cand in (1, 2, 4):
                    if cand < NSETS and np.array_equal(
                            np.tile(mask[:cand], NSETS // cand), mask):
                        q = cand
                        break
                pm = mask[:q]
                ones = [int(c) for c in np.nonzero(pm)[0]]
                zeros = [c for c in range(q) if c not in ones]
                maj, minority = (1.0, zeros) if len(ones) >= len(zeros) \
                    else (0.0, ones)
                eng.memset(t[:], maj)
                t3 = t[:].rearrange("p (r c) -> p r c", c=q)
                for c in minority:
                    # integer index -> squeezed 2D strided AP (3D count-1
                    # APs hard-fault the engines)
                    eng.memset(t3[:, :, c], 1.0 - maj)

            # One store per output; the source AP repeats the small tile via
            # a stride-0 dim. bel on the SP ring, pl on the ACT ring.
            for t, out, eng in ((belt, bel, nc.sync), (plt, pl, nc.scalar)):
                o3 = out[:].rearrange("p (r f) -> p r f", r=rep)
                sap = t[:].unsqueeze(1).broadcast_to([P, rep, src_f])
                eng.dma_start(out=o3, in_=sap)

    nc.finalize()
    return nc


def _get_nc(bel_mask, pl_mask):
    key = (tuple(bel_mask), tuple(pl_mask))
    if key not in _NC_CACHE:
        _NC_CACHE[key] = _build_nc(bel_mask, pl_mask)
    return _NC_CACHE[key]


def kernel(inputs, focal):
    global LAST_RESULTS
    inputs = np.asarray(inputs)
    focal_i = int(np.asarray(focal))
    assert inputs.shape == (B, H, W, C), inputs.shape

    # Host-side mask computation (cheap: 8 elements).
    j = np.arange(NSETS, dtype=np.int64)
    contain = j & focal_i
    bel_mask = (contain == focal_i).astype(np.float32)
    pl_mask = (contain > 0).astype(np.float32)

    nc = _get_nc(bel_mask, pl_mask)
    in_maps = [{} for _ in range(N_CORES)]
    res = run_bass_kernel_spmd(nc, in_maps, list(range(N_CORES)))
    LAST_RESULTS = res

    out_dtype = inputs.dtype
    bel_full = np.empty((B, H, W, NSETS), dtype=out_dtype)
    pl_full = np.empty((B, H, W, NSETS), dtype=out_dtype)
    for b in range(N_CORES):
        bel_full[b] = res.results[b]["bel"].reshape(H, W, NSETS)
        pl_full[b] = res.results[b]["pl"].reshape(H, W, NSETS)
    return (bel_full, pl_full)


# revision 17
# speedup vs baseline: 1.2019x; 1.0080x over previous
"""Trainium2 Bass kernel for nn_BeliefPlausibilityFocused.

reference():
    cardinal_fod = inputs.shape[-1] - 1 = 3; n_sets = 8
    bel[..., j] = 1.0 if (j & focal) == focal else 0.0
    pl[...,  j] = 1.0 if (j & focal) >  0    else 0.0
Both outputs are per-pixel broadcast constants of shape
inputs.shape[:-1] + (8,) = [8, 384, 1248, 8]; the input VALUES are unused.

Strategy (pure data-parallel over batch, per sharding hint):
  - 8 cores, one batch element each. Per-core output: bel/pl each
    [384, 1248, 8] f32 = 15.3 MB -> 30.7 MB of HBM writes per core; no
    inputs are transferred to the device at all.
  - The masks (derived from `focal` on the host) are baked into the program
    as memsets: each 8-periodic pattern is built in a small SBUF tile
    (bulk memset of the majority value + one strided memset per minority
    channel, folded to the mask's minimal period). The memsets are emitted
    in the entry basic block so they overlap the framework preamble;
    cross-engine ordering is by explicit semaphores.
  - Each output is then written by ONE large HWDGE DMA whose source AP
    repeats the small tile via a stride-0 dim (bel on the SP ring, pl on
    the ACT ring).
  - Measured ~86 us/core when HBM is uncontended (~410 GB/s store BW,
    ~94% of the 435 GB/s SBUF-port ceiling); all-core aggregate sits at
    the device HBM write roofline (~245 MB over ~85 us).
"""

import sys
import types

import numpy as np

import concourse.bass as bass
import concourse.mybir as mybir
from concourse.bass_utils import run_bass_kernel_spmd


def _install_ntff_hook_shim():
    """bass_utils imports antenv.axon_hooks when BASS_TRACE=1 under axon, but
    the agent image's antenv package lacks that module (a bare import error
    would crash the run). Provide it, wiring the ctypes NTFF hook when the
    axon .so supports it, else degrading to no tracing."""
    if "antenv.axon_hooks" in sys.modules:
        return
    mod = types.ModuleType("antenv.axon_hooks")
    _slot = [None]
    mod.set_axon_ntff_profile_hook = lambda h: _slot.__setitem__(0, h)
    mod.get_axon_ntff_profile_hook = lambda: _slot[0]
    sys.modules["antenv.axon_hooks"] = mod
    try:
        import antenv

        antenv.axon_hooks = mod
    except Exception:
        pass
    try:
        from trn_agent_boot.trn_boot import _ntff_profile_via_ctypes

        hook = _ntff_profile_via_ctypes("/opt/axon/libaxon_pjrt.so")
        if hook is not None:
            mod.set_axon_ntff_profile_hook(hook)
    except Exception:
        pass  # no profiling available; execution still works


_install_ntff_hook_shim()

# Problem shapes (hardcoded per contract: kernel.py must be self-contained).
B, H, W, C = 8, 384, 1248, 4
NSETS = 1 << (C - 1)          # 8
N_CORES = 8
P = 128                        # SBUF partitions

PIX = H * W                    # 479232 pixels per batch element
PER_OUT = PIX * NSETS          # 3,833,856 f32 per output per core
PER_PART = PER_OUT // P        # 29,952 f32 per partition
SRC_F = 1248                   # source tile width; 4992 B per repeat chunk
REP = PER_PART // SRC_F        # 24 stride-0 repeats per store

assert PER_OUT % P == 0 and PER_PART % NSETS == 0 and SRC_F % NSETS == 0
assert SRC_F * REP == PER_PART

_NC_CACHE = {}
LAST_RESULTS = None  # BassKernelResults of the most recent run (for test.py)


def _memset_plan(mask):
    """(period, majority value, minority channels within one period)."""
    mask = np.asarray(mask, np.float32)
    q = NSETS
    for cand in (1, 2, 4):
        if cand < NSETS and np.array_equal(
                np.tile(mask[:cand], NSETS // cand), mask):
            q = cand
            break
    pm = mask[:q]
    ones = [int(c) for c in np.nonzero(pm)[0]]
    zeros = [c for c in range(q) if c not in ones]
    if len(ones) >= len(zeros):
        return q, 1.0, zeros
    return q, 0.0, ones


def _build_nc(bel_mask, pl_mask, src_f=SRC_F):
    rep = PER_PART // src_f
    assert src_f * rep == PER_PART and src_f % NSETS == 0
    nc = bass.Bass(None, target_bir_lowering=False)

    bel = nc.dram_tensor("bel", [P, PER_PART], mybir.dt.float32,
                         kind="ExternalOutput")
    pl = nc.dram_tensor("pl", [P, PER_PART], mybir.dt.float32,
                        kind="ExternalOutput")

    with (
        nc.sbuf_tensor([P, src_f], mybir.dt.float32) as belt,
        nc.sbuf_tensor([P, src_f], mybir.dt.float32) as plt,
        nc.semaphore() as s_bel,
        nc.semaphore() as s_pl,
        nc.semaphore() as s_dma,
    ):
        # Pattern memsets in the entry BB: they overlap the framework
        # preamble. belt on DVE, plt on GpSimd (parallel engines).
        for t, mask, eng, sem in ((belt, bel_mask, nc.vector, s_bel),
                                  (plt, pl_mask, nc.gpsimd, s_pl)):
            q, maj, minority = _memset_plan(mask)
            ins = eng.memset(t[:], maj)
            t3 = t[:].rearrange("p (r c) -> p r c", c=q)
            for c in minority:
                # integer index -> squeezed 2D strided AP (3D count-1 APs
                # hard-fault the engines)
                ins = eng.memset(t3[:, :, c], 1.0 - maj)
            ins.then_inc(sem, 1)

        with nc.Block() as block:
            @block.sync
            def _(s):
                s.wait_ge(s_bel, 1)
                o3 = bel[:].rearrange("p (r f) -> p r f", r=rep)
                src = belt[:].unsqueeze(1).broadcast_to([P, rep, src_f])
                s.dma_start(out=o3, in_=src).then_inc(s_dma, 16)
                # wait for BOTH stores' data to land before kernel end
                s.wait_ge(s_dma, 32)

            @block.scalar
            def _(sc):
                sc.wait_ge(s_pl, 1)
                o3 = pl[:].rearrange("p (r f) -> p r f", r=rep)
                src = plt[:].unsqueeze(1).broadcast_to([P, rep, src_f])
                sc.dma_start(out=o3, in_=src).then_inc(s_dma, 16)

    nc.finalize()
    return nc


def _get_nc(bel_mask, pl_mask):
    key = (tuple(bel_mask), tuple(pl_mask))
    if key not in _NC_CACHE:
        _NC_CACHE[key] = _build_nc(bel_mask, pl_mask)
    return _NC_CACHE[key]


def kernel(inputs, focal):
    global LAST_RESULTS
    inputs = np.asarray(inputs)
    focal_i = int(np.asarray(focal))
    assert inputs.shape == (B, H, W, C), inputs.shape

    # Host-side mask computation (cheap: 8 elements).
    j = np.arange(NSETS, dtype=np.int64)
    contain = j & focal_i
    bel_mask = (contain == focal_i).astype(np.float32)
    pl_mask = (contain > 0).astype(np.float32)

    nc = _get_nc(bel_mask, pl_mask)
    in_maps = [{} for _ in range(N_CORES)]
    res = run_bass_kernel_spmd(nc, in_maps, list(range(N_CORES)))
    LAST_RESULTS = res

    out_dtype = inputs.dtype
    bel_full = np.empty((B, H, W, NSETS), dtype=out_dtype)
    pl_full = np.empty((B, H, W, NSETS), dtype=out_dtype)
    for b in range(N_CORES):
        bel_full[b] = res.results[b]["bel"].reshape(H, W, NSETS)
        pl_full[b] = res.results[b]["pl"].reshape(H, W, NSETS)
    return (bel_full, pl_full)


# revision 18
# speedup vs baseline: 1.2043x; 1.0021x over previous
"""Trainium2 Bass kernel for nn_BeliefPlausibilityFocused.

reference():
    cardinal_fod = inputs.shape[-1] - 1 = 3; n_sets = 8
    bel[..., j] = 1.0 if (j & focal) == focal else 0.0
    pl[...,  j] = 1.0 if (j & focal) >  0    else 0.0
Both outputs are per-pixel broadcast constants of shape
inputs.shape[:-1] + (8,) = [8, 384, 1248, 8]; the input VALUES are unused.

Strategy (pure data-parallel over batch, per sharding hint):
  - 8 cores, one batch element each. Per-core output: bel/pl each
    [384, 1248, 8] f32 = 15.3 MB -> 30.7 MB of HBM writes per core; no
    inputs are transferred to the device at all.
  - The masks (derived from `focal` on the host) are baked into the program
    as memsets: each 8-periodic pattern is built in a small SBUF tile
    (bulk memset of the majority value + one strided memset per minority
    channel, folded to the mask's minimal period). The memsets are emitted
    in the entry basic block so they overlap the framework preamble;
    cross-engine ordering is by explicit semaphores.
  - Each output is then written by ONE large HWDGE DMA whose source AP
    repeats the small tile via a stride-0 dim (bel on the SP ring, pl on
    the ACT ring).
  - Measured ~86 us/core when HBM is uncontended (~410 GB/s store BW,
    ~94% of the 435 GB/s SBUF-port ceiling); all-core aggregate sits at
    the device HBM write roofline (~245 MB over ~85 us).
"""

import sys
import types

import numpy as np

import concourse.bass as bass
import concourse.mybir as mybir
from concourse.bass_utils import run_bass_kernel_spmd


def _install_ntff_hook_shim():
    """bass_utils imports antenv.axon_hooks when BASS_TRACE=1 under axon, but
    the agent image's antenv package lacks that module (a bare import error
    would crash the run). Provide it, wiring the ctypes NTFF hook when the
    axon .so supports it, else degrading to no tracing."""
    if "antenv.axon_hooks" in sys.modules:
        return
    mod = types.ModuleType("antenv.axon_hooks")
    _slot = [None]
    mod.set_axon_ntff_profile_hook = lambda h: _slot.__setitem__(0, h)
    mod.get_axon_ntff_profile_hook = lambda: _slot[0]
    sys.modules["antenv.axon_hooks"] = mod
    try:
        import antenv

        antenv.axon_hooks = mod
    except Exception:
        pass
    try:
        from trn_agent_boot.trn_boot import _ntff_profile_via_ctypes

        hook = _ntff_profile_via_ctypes("/opt/axon/libaxon_pjrt.so")
        if hook is not None:
            mod.set_axon_ntff_profile_hook(hook)
    except Exception:
        pass  # no profiling available; execution still works


_install_ntff_hook_shim()

# Problem shapes (hardcoded per contract: kernel.py must be self-contained).
B, H, W, C = 8, 384, 1248, 4
NSETS = 1 << (C - 1)          # 8
N_CORES = 8
P = 128                        # SBUF partitions

PIX = H * W                    # 479232 pixels per batch element
PER_OUT = PIX * NSETS          # 3,833,856 f32 per output per core
PER_PART = PER_OUT // P        # 29,952 f32 per partition
SRC_F = 1248                   # source tile width; 4992 B per repeat chunk
REP = PER_PART // SRC_F        # 24 stride-0 repeats per store

assert PER_OUT % P == 0 and PER_PART % NSETS == 0 and SRC_F % NSETS == 0
assert SRC_F * REP == PER_PART

_NC_CACHE = {}
LAST_RESULTS = None  # BassKernelResults of the most recent run (for test.py)


def _memset_plan(mask):
    """(period, majority value, minority channels within one period)."""
    mask = np.asarray(mask, np.float32)
    q = NSETS
    for cand in (1, 2, 4):
        if cand < NSETS and np.array_equal(
                np.tile(mask[:cand], NSETS // cand), mask):
            q = cand
            break
    pm = mask[:q]
    ones = [int(c) for c in np.nonzero(pm)[0]]
    zeros = [c for c in range(q) if c not in ones]
    if len(ones) >= len(zeros):
        return q, 1.0, zeros
    return q, 0.0, ones


def _build_nc(bel_mask, pl_mask, src_f=SRC_F):
    rep = PER_PART // src_f
    assert src_f * rep == PER_PART and src_f % NSETS == 0
    nc = bass.Bass(None, target_bir_lowering=False)

    bel = nc.dram_tensor("bel", [P, PER_PART], mybir.dt.float32,
                         kind="ExternalOutput")
    pl = nc.dram_tensor("pl", [P, PER_PART], mybir.dt.float32,
                        kind="ExternalOutput")

    with (
        nc.sbuf_tensor([P, src_f], mybir.dt.float32) as belt,
        nc.sbuf_tensor([P, src_f], mybir.dt.float32) as plt,
        nc.semaphore() as s_bel,
        nc.semaphore() as s_pl,
        nc.semaphore() as s_dma,
    ):
        # Pattern fills in the entry BB: they overlap the framework
        # preamble. Seed one period with tiny memsets, then replicate it
        # across the tile with a single stride-0-source copy.
        # belt on DVE, plt on GpSimd (parallel engines).
        for t, mask, eng, sem in ((belt, bel_mask, nc.vector, s_bel),
                                  (plt, pl_mask, nc.gpsimd, s_pl)):
            q, maj, minority = _memset_plan(mask)
            eng.memset(t[:, 0:q], maj)
            for c in minority:
                eng.memset(t[:, c:c + 1], 1.0 - maj)
            nper = src_f // q
            dst = t[:].rearrange("p (r c) -> p r c", c=q)[:, 1:]
            src = t[:, 0:q].unsqueeze(1).broadcast_to([P, nper - 1, q])
            eng.tensor_copy(out=dst, in_=src).then_inc(sem, 1)

        with nc.Block() as block:
            @block.sync
            def _(s):
                s.wait_ge(s_bel, 1)
                o3 = bel[:].rearrange("p (r f) -> p r f", r=rep)
                src = belt[:].unsqueeze(1).broadcast_to([P, rep, src_f])
                s.dma_start(out=o3, in_=src).then_inc(s_dma, 16)
                # wait for BOTH stores' data to land before kernel end
                s.wait_ge(s_dma, 32)

            @block.scalar
            def _(sc):
                sc.wait_ge(s_pl, 1)
                o3 = pl[:].rearrange("p (r f) -> p r f", r=rep)
                src = plt[:].unsqueeze(1).broadcast_to([P, rep, src_f])
                sc.dma_start(out=o3, in_=src).then_inc(s_dma, 16)

    nc.finalize()
    return nc


def _get_nc(bel_mask, pl_mask):
    key = (tuple(bel_mask), tuple(pl_mask))
    if key not in _NC_CACHE:
        _NC_CACHE[key] = _build_nc(bel_mask, pl_mask)
    return _NC_CACHE[key]


def kernel(inputs, focal):
    global LAST_RESULTS
    inputs = np.asarray(inputs)
    focal_i = int(np.asarray(focal))
    assert inputs.shape == (B, H, W, C), inputs.shape

    # Host-side mask computation (cheap: 8 elements).
    j = np.arange(NSETS, dtype=np.int64)
    contain = j & focal_i
    bel_mask = (contain == focal_i).astype(np.float32)
    pl_mask = (contain > 0).astype(np.float32)

    nc = _get_nc(bel_mask, pl_mask)
    in_maps = [{} for _ in range(N_CORES)]
    res = run_bass_kernel_spmd(nc, in_maps, list(range(N_CORES)))
    LAST_RESULTS = res

    out_dtype = inputs.dtype
    bel_full = np.empty((B, H, W, NSETS), dtype=out_dtype)
    pl_full = np.empty((B, H, W, NSETS), dtype=out_dtype)
    for b in range(N_CORES):
        bel_full[b] = res.results[b]["bel"].reshape(H, W, NSETS)
        pl_full[b] = res.results[b]["pl"].reshape(H, W, NSETS)
    return (bel_full, pl_full)
